# revision 36
# baseline (speedup 1.0000x reference)
"""Trainium2 Bass kernel for batched multi-head attention.

Problem: query/key/value [B=2, H=16, S=2048, D=64] fp32, per-(b,h) divisor
`inv_scale_factor` [B, H, 1, 1].  out = softmax(Q K^T / inv_scale) V.

Sharding: the 32 (b,h) heads are split across 8 NeuronCores, 4 heads per
core, fully data-parallel (no collectives).  Each core runs the same
program on its own 4-head slice.

Per-core algorithm (per head, Sq tiled into q-blocks of 1024):
  - Load Q, K, V naturally ([128 seq, 64 d] tiles).  Cast to fp16 on DVE
    into a PADDED layout [128, 16*128]: each 128-col block holds one seq
    tile's 64 d-values in cols 0:64 and zeros in cols 64:128 (pads zeroed
    once per pool slot).  The Q cast folds in the per-head 1/inv_scale.
  - Transpose Q and K on the DMA XBAR (dma_start with transpose): one
    [128, 2048] fp16 transpose per head per tensor (14ns per 16x128 xbar
    tile, on the DMA engines -- zero PE/DVE cost).  The padded layout makes
    the XBAR emit qt/kt[d 0:64 | zeros 64:128, seq] directly, so QK^T can
    contract over the full K=128 partitions (keeps the PE clock at 2.4GHz)
    with no extra zeroing.
  - scores_T[kv, q] = kt_tile.T @ qt on the PE (fp16 in, fp32 PSUM).
  - P^T = exp(scores_T - ln 128) on ACT straight out of PSUM, fp16 out.
    ACT is the saturated engine (128 x ~1.1us activates); everything else
    is scheduled around keeping its queue fed.
  - PV uses V augmented with a ones column ([kv, 65] fp16 stationary), so
    the softmax denominator (row 64) falls out of the same accumulating
    matmul chain.  The [65, 1024] accumulator pool is double-buffered (4
    PSUM banks + 4 for scores = all 8).
  - Epilogue: accumulator -> fp16 SBUF (DVE), transpose back on the XBAR
    ([80, 1024] -> [128, 8x80]), then per q-tile reciprocal + scale on DVE
    (all-SBUF) and store.
"""

import numpy as np

import concourse.bass as bass
import concourse.tile as tile
from concourse import bacc, mybir
from concourse.bass_utils import run_bass_kernel_spmd
from concourse.masks import make_identity

F32 = mybir.dt.float32
F16 = mybir.dt.float16
EXP = mybir.ActivationFunctionType.Exp
LNP = float(np.log(128.0))

B, H, SQ, SKV, D = 2, 16, 2048, 2048, 64
N_CORES = 8
HEADS_PER_CORE = (B * H) // N_CORES  # 4


def build_attention(nh=HEADS_PER_CORE, sq=SQ, skv=SKV, d=D, qblock=1024,
                    num_devices=N_CORES, enable_asserts=False):
    """Build the per-core Bass program. Returns the compiled Bacc module."""
    assert d == 64
    assert sq % 128 == 0 and skv % 128 == 0
    qblock = min(qblock, sq)
    assert sq % qblock == 0
    nchunk = min(512, qblock)          # matmul moving free-dim chunk
    assert qblock % nchunk == 0
    ntq = sq // 128                    # q tiles per head
    nkv = skv // 128                   # kv tiles per head
    nqb = sq // qblock                 # q blocks per head
    ntq_b = qblock // 128              # q tiles per q block

    nc = bacc.Bacc("TRN2", target_bir_lowering=False, debug=False,
                   enable_asserts=enable_asserts, num_devices=num_devices)

    q_dram = nc.dram_tensor("query", [nh, sq, d], F32, kind="ExternalInput").ap()
    k_dram = nc.dram_tensor("key", [nh, skv, d], F32, kind="ExternalInput").ap()
    v_dram = nc.dram_tensor("value", [nh, skv, d], F32, kind="ExternalInput").ap()
    inv_dram = nc.dram_tensor("inv_scale", [1, nh], F32, kind="ExternalInput").ap()
    o_dram = nc.dram_tensor("out", [nh, sq, d], F32, kind="ExternalOutput").ap()

    with tile.TileContext(nc) as tc:
        _attention_body(tc, o_dram, q_dram, k_dram, v_dram, inv_dram,
                        nh, sq, skv, d, qblock, nchunk, ntq, nkv, nqb, ntq_b)

    nc.compile()
    return nc


def _attention_body(tc, o_dram, q_dram, k_dram, v_dram, inv_dram,
                    nh, sq, skv, d, qblock, nchunk, ntq, nkv, nqb, ntq_b):
    nc = tc.nc
    from contextlib import ExitStack
    with ExitStack() as ctx:
        const = ctx.enter_context(tc.tile_pool(name="const", bufs=1))
        qnatp = ctx.enter_context(tc.tile_pool(name="qnat", bufs=2))
        knatp = ctx.enter_context(tc.tile_pool(name="knat", bufs=2))
        vnatp = ctx.enter_context(tc.tile_pool(name="vnat", bufs=2))
        qhp = ctx.enter_context(tc.tile_pool(name="qh", bufs=2))
        khp = ctx.enter_context(tc.tile_pool(name="kh", bufs=2))
        qtp = ctx.enter_context(tc.tile_pool(name="qt", bufs=2))
        ktp = ctx.enter_context(tc.tile_pool(name="kt", bufs=2))
        vaugp = ctx.enter_context(tc.tile_pool(name="vaug", bufs=2))
        ptp = ctx.enter_context(tc.tile_pool(name="pt", bufs=6))
        osbp = ctx.enter_context(tc.tile_pool(name="osb", bufs=2))
        psop = ctx.enter_context(tc.tile_pool(name="pso", bufs=2))
        finp = ctx.enter_context(tc.tile_pool(name="fin", bufs=2))
        recp = ctx.enter_context(tc.tile_pool(name="rec", bufs=4))
        scp = ctx.enter_context(tc.tile_pool(name="scps", bufs=2, space="PSUM"))
        outp = ctx.enter_context(tc.tile_pool(name="outps", bufs=2, space="PSUM"))

        # --- constants: exp bias column, per-head 1/inv_scale [128, nh].
        # The DMA broadcasts the [1, nh] divisor row to all 128 partitions
        # (stride-0 source AP), so no PE/PSUM round-trip is needed.
        # fp16 identity for the head-0 ramp PE transposes (the XBAR path has
        # a completion-vs-consumption race when the consumer follows within
        # ~1us; the ramp prefix is the only such case, so it goes on the PE).
        # Built directly in fp16 on gpsimd so it's ready ~8us with no DVE
        # dependency (the warmup matmuls gate on it).
        ident_h = const.tile([128, 128], F16)
        make_identity(nc, ident_h[:])
        bias_col = const.tile([128, 1], F32)
        nc.vector.memset(bias_col[:], -LNP)
        inv_bc = const.tile([128, nh], F32)
        nc.sync.dma_start(inv_bc[:], inv_dram[0, :].partition_broadcast(128))
        scale_all = const.tile([128, nh], F32)
        nc.vector.reciprocal(scale_all[:], inv_bc[:])

        NKR = 8  # kv tiles PE-transposed in the ramp prefix (head 0)

        def stage_head_loads(h, ramp=False):
            """DMA issues + fp16 pack + transposes for head h.  The casts
            and XBAR transposes are returned as closures drained into the
            main loop's slack (consumed a full head later -- huge margin).
            For the ramp (head 0), q-block 0 of Q and the first NKR kv
            tiles are transposed on the PE (identity matmul) inline, since
            their consumers follow too closely for the XBAR."""
            qnat = qnatp.tile([128, ntq * d], F32, tag="qnat", name="qnat")
            qdr = q_dram[h].rearrange("(t p) e -> p t e", p=128)
            qnv = qnat[:].rearrange("p (t e) -> p t e", e=d)
            knat = knatp.tile([128, nkv * d], F32, tag="knat", name="knat")
            kdr = k_dram[h].rearrange("(t p) e -> p t e", p=128)
            knv = knat[:].rearrange("p (t e) -> p t e", e=d)
            vnat = vnatp.tile([128, nkv * (d + 1)], F32, tag="vnat", name="vnat")
            nc.gpsimd.memset(vnat[:], 1.0)
            vnv = vnat[:].rearrange("p (t e) -> p t e", e=d + 1)
            vdr = v_dram[h].rearrange("(t p) e -> p t e", p=128)

            # DMA issue order: the ramp puts K 0:2 and q-block-0 pieces
            # first in small 2-tile pieces (land earlier -- they gate the
            # first QK); steady state is 4-tile pieces.
            if ramp:
                nc.sync.dma_start(knv[:, 0:2, :], kdr[:, 0:2, :])
                nc.sync.dma_start(knv[:, 2:6, :], kdr[:, 2:6, :])
                for j in range(0, ntq_b, 2):
                    nc.sync.dma_start(qnv[:, j:j + 2, :], qdr[:, j:j + 2, :])
                nc.sync.dma_start(vnv[:, 0:4, 0:d], vdr[:, 0:4, :])
                nc.sync.dma_start(knv[:, 6:10, :], kdr[:, 6:10, :])
                for j in range(4, nkv, 4):
                    nc.sync.dma_start(vnv[:, j:j + 4, 0:d], vdr[:, j:j + 4, :])
                nc.sync.dma_start(knv[:, 10:nkv, :], kdr[:, 10:nkv, :])
                for j in range(ntq_b, ntq, 4):
                    nc.sync.dma_start(qnv[:, j:j + 4, :], qdr[:, j:j + 4, :])
            else:
                # K in 4-tile pieces (lands first, feeds the earliest
                # drains); Q/V as big 8-tile pieces to keep the DMA issue
                # count low (each issue occupies one of ~9 rotating DMA
                # semaphores until it completes -- too many pieces stall
                # the in-order SP queue on semaphore recycling)
                for j in range(0, nkv, 4):
                    nc.sync.dma_start(knv[:, j:j + 4, :], kdr[:, j:j + 4, :])
                for j in range(0, ntq, 8):
                    nc.sync.dma_start(qnv[:, j:j + 8, :], qdr[:, j:j + 8, :])
                for j in range(0, nkv, 8):
                    nc.sync.dma_start(vnv[:, j:j + 8, 0:d], vdr[:, j:j + 8, :])

            # fp16 staging, PADDED: 128-col block per tile, data in 0:64.
            qh16 = qhp.tile([128, ntq * 128], F16, tag="qh", name="qh16")
            kh16 = khp.tile([128, nkv * 128], F16, tag="kh", name="kh16")
            qhv = qh16[:].rearrange("p (t x) -> p t x", x=128)
            khv = kh16[:].rearrange("p (t x) -> p t x", x=128)
            if h < 2:
                # zero the pad columns once per pool slot (they become qt/kt
                # rows 64:128 through the XBAR, enabling K=128 contraction)
                nc.gpsimd.memset(qhv[:, :, d:128], 0.0)
                nc.gpsimd.memset(khv[:, :, d:128], 0.0)
            sh = scale_all[:, h:h + 1]
            vaug = vaugp.tile([128, nkv * (d + 1)], F16, tag="vaug", name="vaug")
            qt = qtp.tile([128, sq], F16, tag="qt", name="qt")
            kt = ktp.tile([128, skv], F16, tag="kt", name="kt")
            qtv = qt[:].rearrange("p (t q) -> p t q", q=128)
            ktv = kt[:].rearrange("p (t q) -> p t q", q=128)

            def kc(t0, t1):
                nc.vector.tensor_copy(khv[:, t0:t1, 0:d], knv[:, t0:t1, :])

            def kx(t0, t1):
                nc.sync.dma_start_transpose(ktv[:, t0:t1, :],
                                            kh16[:, t0 * 128:t1 * 128])

            def qc(t0, t1):
                nc.vector.tensor_scalar_mul(qhv[:, t0:t1, 0:d], qnv[:, t0:t1, :], sh)

            def qx(t0, t1):
                nc.sync.dma_start_transpose(qtv[:, t0:t1, :],
                                            qh16[:, t0 * 128:t1 * 128])

            def vc(t0, t1):
                nc.vector.tensor_copy(vaug[:, t0 * (d + 1):t1 * (d + 1)],
                                      vnat[:, t0 * (d + 1):t1 * (d + 1)])

            if ramp:
                # PE-transpose prefix: zero rows 64:128 of the PE-written
                # regions (XBAR-written regions get zeros via the pads).
                # The qb0 casts are PLAIN (no scale) so they start the
                # moment Q lands; the per-head 1/inv_scale is applied by
                # the transpose copybacks instead (scale operand), which
                # decouples the cast chain from the inv broadcast DMA.
                # Copybacks alternate DVE / ACT (ACT is idle pre-exp).
                nc.gpsimd.memset(qt[64:128, 0:qblock], 0.0)
                nc.gpsimd.memset(kt[64:128, 0:NKR * 128], 0.0)
                nc.vector.tensor_copy(qhv[:, 0:4, 0:d], qnv[:, 0:4, :])
                nc.vector.tensor_copy(qhv[:, 4:ntq_b, 0:d], qnv[:, 4:ntq_b, :])
                kc(0, 2)
                kc(2, 6)
                tpA = scp.tile([128, qblock], F32, tag="sc", name="tpA")
                tpB = scp.tile([128, qblock], F32, tag="sc", name="tpB")
                sh64 = scale_all[0:64, h:h + 1]
                # a few dummy full-row matmuls to lift the PE clock gate out
                # of its cold pstate before the real ramp transposes arrive
                for _ in range(4):
                    nc.tensor.matmul(tpA[0:64, 896:1024], ident_h[0:128, 0:64],
                                     ident_h[0:128, 0:128], start=True, stop=True)

                # transposes alternate between tpA/tpB at disjoint column
                # regions: the tile framework tracks PSUM WAR at tile
                # granularity, so a single scratch tile would serialize
                # each matmul behind the previous tile's copyback
                def tq(t):
                    tp = tpA if t % 2 == 0 else tpB
                    nc.tensor.matmul(tp[0:64, t * 128:(t + 1) * 128],
                                     qh16[:, t * 128:t * 128 + d],
                                     ident_h[0:128, 0:128], start=True, stop=True)
                    if t % 2 == 0:
                        nc.vector.tensor_scalar_mul(
                            qt[0:64, t * 128:(t + 1) * 128],
                            tp[0:64, t * 128:(t + 1) * 128], sh64)
                    else:
                        nc.scalar.mul(qt[0:64, t * 128:(t + 1) * 128],
                                      tp[0:64, t * 128:(t + 1) * 128], sh64)

                def tk(t):
                    tp = tpB if t % 2 == 0 else tpA
                    nc.tensor.matmul(tp[0:64, t * 128:(t + 1) * 128],
                                     kh16[:, t * 128:t * 128 + d],
                                     ident_h[0:128, 0:128], start=True, stop=True)
                    if t % 2 == 0:
                        nc.vector.tensor_copy(kt[0:64, t * 128:(t + 1) * 128],
                                              tp[0:64, t * 128:(t + 1) * 128])
                    else:
                        nc.scalar.copy(kt[0:64, t * 128:(t + 1) * 128],
                                       tp[0:64, t * 128:(t + 1) * 128])

                # inline only what QK(0) needs (qt q-block 0 + kt tiles 0-1);
                # kt 2..7 drain as closures so they interleave into the PE
                # stream between early QKs instead of gating QK(0) behind
                # the later K DMA pieces (the PE queue is in-order)
                for t in range(ntq_b):
                    tq(t)
                for t in range(2):
                    tk(t)
                vc(0, 4)
                vc(4, 10)
                vc(10, nkv)
                closures = [
                    (0, lambda: kc(6, 10)), (0, lambda: tk(2)),
                    (1, lambda: tk(3)), (1, lambda: tk(4)),
                    (2, lambda: tk(5)), (2, lambda: tk(6)),
                    (3, lambda: tk(7)), (3, lambda: kc(10, nkv)),
                    (4, lambda: kx(NKR, nkv)), (5, lambda: qc(ntq_b, ntq)),
                    (6, lambda: qx(ntq_b, ntq)),
                ]
            else:
                # min-iteration gates: each cast drains only after its DMA
                # has landed, so a closure never head-of-line-blocks the
                # in-order DVE queue behind a multi-us DMA wait
                closures = [
                    (12, lambda: kc(0, nkv // 2)),
                    (13, lambda: kc(nkv // 2, nkv)),
                    (14, lambda: kx(0, nkv)),
                    (16, lambda: qc(0, ntq_b)), (17, lambda: qc(ntq_b, ntq)),
                    (18, lambda: qx(0, ntq)),
                    (19, lambda: vc(0, nkv // 2)),
                    (20, lambda: vc(nkv // 2, nkv)),
                ]
            return qt, kt, vaug, closures

        staged = stage_head_loads(0, ramp=True)

        osb_count = [0]

        def make_epilogue(h, qb, out_ps, last=False):
            """Per-q-block epilogue closures, drained one per kv-iteration.
            Transpose-back runs on the DMA XBAR ([80, 1024] fp16 in two
            halves); normalization is all-SBUF on the DVE."""
            cell = {}
            npso = 80  # rows per tile in the transposed staging (64d+denom+pad)

            def c_copy(half):
                if half == 0:
                    osb = osbp.tile([128, qblock], F16, tag="osb", name="osb")
                    if osb_count[0] < 2:
                        # rows 65:80 are read by the XBAR and 65:128 by the
                        # last-block PE transpose; zero once per slot (the
                        # copy below rewrites row 64 with the denominator)
                        nc.vector.memset(osb[64:128, :], 0.0)
                    osb_count[0] += 1
                    cell["osb"] = osb
                c0 = half * (qblock // 2)
                c1 = c0 + qblock // 2
                nc.vector.tensor_copy(cell["osb"][0:65, c0:c1], out_ps[0:65, c0:c1])

            def c_xbar(half):
                if half == 0:
                    pso = psop.tile([128, ntq_b * npso], F16, tag="pso", name="pso")
                    cell["pso"] = pso
                c0 = half * (qblock // 2)
                c1 = c0 + qblock // 2
                t0 = half * (ntq_b // 2)
                t1 = t0 + ntq_b // 2
                pv = cell["pso"][:].rearrange("p (t r) -> p t r", r=npso)
                # the last epilogue's first XBAR goes on the then-idle ACT
                # queue so the two ~1.2us issues run in parallel (SP + ACT)
                xq = nc.scalar if (last and half == 0) else nc.sync
                xq.dma_start_transpose(pv[:, t0:t1, :], cell["osb"][0:npso, c0:c1])

            def c_tile(st):
                if st == 0:
                    cell["fin"] = finp.tile([128, ntq_b * d], F32, tag="fin",
                                            name="fin")
                pso = cell["pso"]
                rec = recp.tile([128, 1], F32, tag="rec", name="rec")
                nc.vector.reciprocal(rec[:], pso[:, st * npso + d:st * npso + d + 1])
                nc.vector.tensor_scalar_mul(
                    cell["fin"][:, st * d:(st + 1) * d],
                    pso[:, st * npso:st * npso + d], rec[:])

            def c_tile_pe(st):
                # last-q-block path: transpose back on the now-idle PE
                # (matmul vs identity into a free scores PSUM slot) instead
                # of the XBAR, whose ~3us issue+completion latency would
                # sit exposed on the tail's critical path.
                if st == 0:
                    cell["fin"] = finp.tile([128, ntq_b * d], F32, tag="fin",
                                            name="fin")
                    # two scratch tiles: tile-granular PSUM WAR tracking
                    # would otherwise serialize each transpose behind the
                    # previous tile's normalization reads
                    cell["tp0"] = scp.tile([128, qblock], F32, tag="sc",
                                           name="tp_epi0")
                    cell["tp1"] = scp.tile([128, qblock], F32, tag="sc",
                                           name="tp_epi1")
                tp = cell["tp0"] if st % 2 == 0 else cell["tp1"]
                nc.tensor.matmul(tp[0:128, st * 128:st * 128 + d + 1],
                                 cell["osb"][0:128, st * 128:(st + 1) * 128],
                                 ident_h[0:128, 0:d + 1], start=True, stop=True)
                rec = recp.tile([128, 1], F32, tag="rec", name="rec")
                nc.vector.reciprocal(rec[:], tp[:, st * 128 + d:st * 128 + d + 1])
                nc.vector.tensor_scalar_mul(
                    cell["fin"][:, st * d:(st + 1) * d],
                    tp[:, st * 128:st * 128 + d], rec[:])

            def c_dma(p0, p1):
                odr = o_dram[h].rearrange("(t p) e -> p t e", p=128)
                fv = cell["fin"][:].rearrange("p (t e) -> p t e", e=d)
                for j in range(p0, p1, 2):
                    nc.sync.dma_start(
                        odr[:, qb * ntq_b + j:qb * ntq_b + j + 2, :],
                        fv[:, j:j + 2, :])

            # the PSUM->SBUF copies run inline (not queued): they release
            # the accumulator slot, which the next q-blocks transitively
            # wait on; queue only the latency-tolerant rest
            c_copy(0)
            c_copy(1)
            if last:
                eps = []
                for st in range(ntq_b):
                    eps.append(lambda st=st: c_tile_pe(st))
                    if st % 2 == 1:
                        eps.append(lambda j=st: c_dma(j - 1, j + 1))
            else:
                eps = [lambda: c_xbar(0), lambda: c_xbar(1)]
                for st in range(ntq_b):
                    eps.append(lambda st=st: c_tile(st))
                    if st % 2 == 1:
                        eps.append(lambda j=st: c_dma(j - 1, j + 1))
            return eps

        # ---------------- main loops ----------------
        # Per head, a flat (qb, kv) stream, software-pipelined in emission:
        #   QK(i+1), exp(i), PV(i)
        # so the in-order PE always has the next scores matmul queued while
        # ACT runs exp(i); ACT is the saturated engine.  Background `work`
        # (next head's staging, previous q-block's epilogue) is drained a
        # bit per iteration into the PE/DVE/SP slack.
        stage_q = []   # next head's staging: MUST be empty before that head
        epi_q = []     # epilogue pieces: only self-dependent, may trail
        niter = nqb * nkv
        out_ps = None
        for h in range(nh):
            qt, kt, vaug, pending = staged
            stage_q.extend(pending)
            nxt = None

            def emit_qk(it):
                qb, kvt = divmod(it, nkv)
                q0 = qb * qblock
                sc = scp.tile([128, qblock], F32, tag="sc", name="sc")
                for c in range(qblock // nchunk):
                    nc.tensor.matmul(
                        sc[:, c * nchunk:(c + 1) * nchunk],
                        kt[0:128, kvt * 128:(kvt + 1) * 128],
                        qt[0:128, q0 + c * nchunk:q0 + (c + 1) * nchunk],
                        start=True, stop=True)
                return sc

            def exp_imm(out, in_):
                # Exp with IMMEDIATE bias/scale: bass force-converts float
                # biases to an SBUF AP for table funcs, and the per-ACTIVATE
                # AP read costs ~80ns; emit the raw instruction instead.
                se = nc.scalar
                ins_ = [se.lower_ap(in_),
                        mybir.ImmediateValue(dtype=mybir.dt.float32, value=-LNP),
                        mybir.ImmediateValue(dtype=mybir.dt.float32, value=1.0),
                        mybir.ImmediateValue(dtype=mybir.dt.float32, value=0.0)]
                return se.add_instruction(mybir.InstActivation(
                    name=se.bass.get_next_instruction_name(),
                    func=EXP, ins=ins_, outs=[se.lower_ap(out)]))

            def emit_exp(it, sc, split):
                pt = ptp.tile([128, qblock], F16, tag="pt")
                if split:
                    # head-0 iteration 0: two half-width activates so the
                    # first exp starts as soon as QK chunk 0 lands
                    hw_ = qblock // 2
                    exp_imm(pt[:, 0:hw_], sc[:, 0:hw_])
                    exp_imm(pt[:, hw_:], sc[:, hw_:])
                else:
                    exp_imm(pt[:], sc[:])
                return pt

            def emit_pv(pit, ppt):
                # PV for iteration pit, emitted one iteration late so the
                # in-order PE starts the next QK (which gates the next EXP
                # via the double-buffered score slots) first.
                nonlocal out_ps
                pqb, pkv = divmod(pit, nkv)
                if pkv == 0:
                    out_ps = outp.tile([65, qblock], F32, tag="out",
                                       name="out_ps")
                for c in range(qblock // nchunk):
                    nc.tensor.matmul(
                        out_ps[0:65, c * nchunk:(c + 1) * nchunk],
                        vaug[:, pkv * (d + 1):(pkv + 1) * (d + 1)],
                        ppt[:, c * nchunk:(c + 1) * nchunk],
                        start=(pkv == 0), stop=(pkv == nkv - 1))
                if pkv == nkv - 1:
                    last = (h == nh - 1) and (pqb == nqb - 1)
                    epi_q.extend(make_epilogue(h, pqb, out_ps, last=last))

            sc_cur = emit_qk(0)
            prev_pt = None
            for it in range(niter):
                if it == 8 and h + 1 < nh:
                    # at it==8 the ramp/previous-head closures have drained,
                    # so the 12 bulk DMA issues can't head-of-line block a
                    # time-critical XBAR issue on the SP queue
                    nxt = stage_head_loads(h + 1)
                    stage_q.extend(nxt[3])
                sc_next = emit_qk(it + 1) if it + 1 < niter else None
                pt = emit_exp(it, sc_cur, split=(h == 0 and it == 0))
                if prev_pt is not None:
                    emit_pv(it - 1, prev_pt)
                prev_pt = pt
                sc_cur = sc_next
                budget = 3
                if epi_q:
                    epi_q.pop(0)()
                    budget -= 1
                while budget > 1 and stage_q and stage_q[0][0] <= it:
                    stage_q.pop(0)[1]()
                    budget -= 1
                while budget and epi_q:
                    epi_q.pop(0)()
                    budget -= 1
            emit_pv(niter - 1, prev_pt)
            while stage_q:
                stage_q.pop(0)[1]()
            if nxt is not None:
                staged = nxt[:3] + ([],)

        while epi_q:
            epi_q.pop(0)()


_NC_CACHE = {}


def _get_program():
    key = "full"
    if key not in _NC_CACHE:
        _NC_CACHE[key] = build_attention()
    return _NC_CACHE[key]


def kernel(query, key, value, inv_scale_factor):
    """Full-input entry point: shard over 8 cores, run, gather."""
    nc = _get_program()
    q = np.ascontiguousarray(query, dtype=np.float32).reshape(B * H, SQ, D)
    k = np.ascontiguousarray(key, dtype=np.float32).reshape(B * H, SKV, D)
    v = np.ascontiguousarray(value, dtype=np.float32).reshape(B * H, SKV, D)
    inv = np.ascontiguousarray(inv_scale_factor, dtype=np.float32).reshape(B * H)

    hpc = HEADS_PER_CORE
    in_maps = []
    for c in range(N_CORES):
        s = slice(c * hpc, (c + 1) * hpc)
        in_maps.append({
            "query": q[s],
            "key": k[s],
            "value": v[s],
            "inv_scale": inv[s].reshape(1, hpc),
        })
    res = run_bass_kernel_spmd(nc, in_maps, core_ids=list(range(N_CORES)))
    out = np.concatenate([res.results[c]["out"] for c in range(N_CORES)], axis=0)
    return out.reshape(B, H, SQ, D)
